# revision 20
# baseline (speedup 1.0000x reference)
"""CodeSwitchLoss Trainium2 kernel (8-core data-parallel).

Math (see reference): V = l2norm rows of the stack [e, k, etk, kte] (4096 x 1024),
S = V @ V.T, E = exp(10*S).
Per anchor row r=(a,i):
  rowsum[r]   = sum_c E[r,c]
  d_b[r]      = E[r, col(b,i)]  (same-sample entries, b=0..3)
  pos[r]      = sum_{b != a} d_b[r]
  denom[r]    = rowsum[r] - d_a[r]          (= pos + neg)
  contrastive = log(denom) - log(pos)
plus cs regularization on normalized rows; total = (sum contrastive + 0.5*sum reg)/B.

Sharding: batch samples split 8 ways. Each core gets the full embedding set,
rolled so its own 128 samples come first; it computes the 512 anchor rows
(4 versions x 128 samples) against all 4096 columns. Scalar partials summed on
host. The roll makes all per-core slice offsets compile-time constants, so one
NEFF serves all 8 cores.

The host ships rows already l2-normalized (scaled by 32 so fp8e4m3 keeps its
relative precision) and pre-transposed to the matmul layout, so the device does
no norm computation at all: fp8 DoubleRow matmuls into PSUM, one exp per
4-bank group (scale folds in the 10/32^2 temperature factor), DVE rowsum
reduces + eye-masked diagonal extraction, and a short log/sqrt tail. The
measured diagonal is subtracted from both rowsum and pos, so the fp8 norm
noise on the huge self term cancels exactly.
"""

import numpy as np
import ml_dtypes

B = 1024
D = 1024
P = 128
NV = 4
NC_CORES = 8
CHUNK = B // NC_CORES  # 128 samples per core
KCH = D // P  # 8 k-chunks
NT = 512  # matmul free-dim tile (one PSUM bank)
SCALE = 32.0  # fp8 pre-scale on normalized rows
EXPS = 10.0 / (SCALE * SCALE)  # exp scale: 1/T divided by SCALE^2

_compiled = {}


def _sched():
    """Pair-half processing order matching W-version DMA arrival.

    Returns a list of (a, v, h): anchor version a (stationary, own 128
    samples), moving version v, column half h. (a,v,h) needs W[v] half h
    and W[a] cols 0:128 (inside h0)."""
    s = []
    for v in range(NV):
        for a in range(v + 1):
            s.append((a, v, 0))
        for b in range(v):
            s.append((v, b, 0))
            s.append((v, b, 1))
        for a in range(v + 1):
            s.append((a, v, 1))
    return s


def _build_kernel(warm=24, warm_free=128, groups_of=4, drow=True,
                  ps_bufs=2):
    from contextlib import ExitStack

    import concourse.bass as bass
    import concourse.tile as tile
    from concourse import bacc, mybir

    fp32 = mybir.dt.float32
    bf16 = mybir.dt.bfloat16
    fp8 = mybir.dt.float8e4
    AX = mybir.AxisListType
    ALU = mybir.AluOpType
    ACTF = mybir.ActivationFunctionType

    nc = bacc.Bacc(
        "TRN2",
        target_bir_lowering=False,
        debug=False,
        enable_asserts=False,
        num_devices=NC_CORES,
    )
    # pre-transposed normalized*32 fp8 embeddings: embT[v*D + d, s] = W_v[s, d]
    embT = nc.dram_tensor("embT", [NV * D, B], fp8, kind="ExternalInput").ap()
    # natural-layout normalized*32 rows of this core's own chunk (for cs reg)
    csrows = nc.dram_tensor("csrows", [NV * P, D], fp8, kind="ExternalInput").ap()
    ratios = nc.dram_tensor("ratios", [P, 1], fp32, kind="ExternalInput").ap()
    eye_d = nc.dram_tensor("eye", [P, P], bf16, kind="ExternalInput").ap()
    out_d = nc.dram_tensor("out", [P, 1], fp32, kind="ExternalOutput").ap()

    sched = _sched()
    ngroups = len(sched) // groups_of
    # h0 slot index per pair (for diag extraction)
    h0_slot = {}
    for slot, (a, v, h) in enumerate(sched):
        if h == 0:
            h0_slot[(a, v)] = slot

    with tile.TileContext(nc) as tc, ExitStack() as ctx:
        consts = ctx.enter_context(tc.tile_pool(name="consts", bufs=1))
        wpool = ctx.enter_context(tc.tile_pool(name="w", bufs=1))
        psum_p = ctx.enter_context(
            tc.tile_pool(name="psum", bufs=ps_bufs, space="PSUM")
        )
        esb_p = ctx.enter_context(tc.tile_pool(name="esb", bufs=3))
        csx_p = ctx.enter_context(tc.tile_pool(name="csx", bufs=1))
        scr_p = ctx.enter_context(tc.tile_pool(name="scr", bufs=2))
        dscr_p = ctx.enter_context(tc.tile_pool(name="dscr", bufs=3))
        rsg_p = ctx.enter_context(tc.tile_pool(name="rsg", bufs=1))
        fin_p = ctx.enter_context(tc.tile_pool(name="fin", bufs=1))

        # Pre-load the activation table set holding BOTH Exp and Ln, so the
        # compiler's table-load pass never has to insert a (1.3us) reload
        # between the exp stream and the log/sqrt tail.
        from concourse.hw_specs import get_activation_tables

        tabs = list(get_activation_tables(nc.m.arch).values())
        set_id = next(
            i for i, s in enumerate(tabs)
            if ACTF.Exp in s and ACTF.Ln in s
        )
        nc.scalar.add_instruction(
            mybir.InstLoadActFuncSet(
                name=nc.scalar.bass.get_next_instruction_name(),
                ins=[], outs=[], act_func_set_id=set_id,
            )
        )

        # --- constants / warmup (PE busy from t=0 so the clock is ramped
        # to max by the time the first real matmul's W tile has landed) ---
        ones_sb = consts.tile([P, warm_free], bf16, tag="ones")
        nc.vector.memset(ones_sb, 1.0)
        eye_sb = consts.tile([P, P], bf16, tag="eye")
        nc.gpsimd.dma_start(out=eye_sb, in_=eye_d)
        r_sb = consts.tile([P, 1], fp32, tag="ratios")
        nc.gpsimd.dma_start(out=r_sb, in_=ratios)

        W = [
            wpool.tile([P, KCH, B], fp8, tag=f"w{v}", name=f"w{v}")
            for v in range(NV)
        ]

        # --- input DMAs (SP/HWDGE, in arrival order the schedule expects).
        # csrows goes first: it is small (fp8) and unblocks the cs-reg chain,
        # which then runs on DVE/Pool during their otherwise-idle prologue.
        # W3's stationary slice (cols 0:128) is pulled forward so (3,b,*)
        # pairs unlock as soon as their moving version is resident.
        csx = csx_p.tile([P, NV, D], fp8, tag="csx")
        nc.sync.dma_start(
            out=csx, in_=csrows.rearrange("(v p) d -> p v d", p=P)
        )

        def w_dma(v, c0, c1):
            nc.sync.dma_start(
                out=W[v][:, :, c0:c1],
                in_=embT[v * D : (v + 1) * D, c0:c1].rearrange(
                    "(mm p) s -> p mm s", p=P
                ),
            )

        w_dma(0, 0, NT)
        w_dma(3, 0, P)
        w_dma(0, NT, B)
        w_dma(1, 0, NT)
        w_dma(1, NT, B)
        w_dma(2, 0, NT)
        w_dma(2, NT, B)
        w_dma(3, P, NT)
        w_dma(3, NT, B)

        # warmup matmuls: no data deps beyond the memset; overwritten later
        warm_ps = psum_p.tile([P, groups_of, NT], fp32, tag="ps", name="ps_w")
        for i in range(warm):
            nc.tensor.matmul(
                warm_ps[:, i % groups_of, 0:warm_free],
                ones_sb, ones_sb, start=True, stop=True,
            )

        # ---- cs regularization on own chunk (emitted up-front so it runs
        # in the DVE/Pool prologue idle window; branch 1 on DVE, branch 2 on
        # Pool). Rows are normalized*32 fp8; the /32 is folded into the
        # sqrt's exp bias below. ----
        e0, k0, etk0, kte0 = (csx[:, vv, :] for vv in range(NV))
        sspack = fin_p.tile([P, 2], fp32, tag="sspack")
        t1 = scr_p.tile([P, D], bf16, tag="cs_t")
        nc.gpsimd.tensor_sub(t1, e0, k0)
        u = scr_p.tile([P, D], bf16, tag="cs_u")
        nc.vector.tensor_scalar_mul(u, t1, r_sb)
        d1 = scr_p.tile([P, D], bf16, tag="cs_t")
        nc.gpsimd.tensor_sub(d1, etk0, k0)
        d1m = scr_p.tile([P, D], bf16, tag="cs_d")
        nc.gpsimd.tensor_sub(d1m, d1, u)
        d2 = scr_p.tile([P, D], bf16, tag="cs_t2")
        nc.gpsimd.tensor_sub(d2, kte0, e0)
        d2m = scr_p.tile([P, D], bf16, tag="cs_d2")
        nc.gpsimd.tensor_add(d2m, d2, u)

        def emit_dsq(which):
            # emitted at group boundaries matching when the Pool-side diff
            # chain actually lands, so the scheduler doesn't order them
            # ahead of main-loop DVE work
            src = d1m if which == 0 else d2m
            dsq = scr_p.tile([P, D], bf16, tag=f"cs_q{which}")
            nc.vector.scalar_tensor_tensor(
                out=dsq, in0=src, scalar=1.0, in1=src,
                op0=ALU.mult, op1=ALU.mult,
                accum_out=sspack[:, which : which + 1],
            )

        # --- main loop: groups of `groups_of` pair-halves ---
        # rowsum partials accumulate per-anchor in bf16 via TensorTensor
        # adds (the only DVE op class with a fast mode); diag extraction
        # runs on the otherwise-idle Pool engine.
        acc = {}
        dvals = fin_p.tile([P, NV * NV], fp32, tag="dvals")  # [:, a*NV + v]

        for g in range(ngroups):
            chunk = sched[g * groups_of : (g + 1) * groups_of]
            ps = psum_p.tile([P, groups_of, NT], fp32, tag="ps", name="ps")
            for j, (a, v, h) in enumerate(chunk):
                if drow:
                    for m in range(0, KCH, 2):
                        nc.tensor.matmul(
                            ps[:, j, :],
                            W[a][:, m : m + 2, 0:P],
                            W[v][:, m : m + 2, h * NT : (h + 1) * NT],
                            start=(m == 0),
                            stop=(m == KCH - 2),
                            perf_mode=mybir.MatmulPerfMode.DoubleRow,
                        )
                else:
                    for m in range(KCH):
                        nc.tensor.matmul(
                            ps[:, j, :],
                            W[a][:, m, 0:P],
                            W[v][:, m, h * NT : (h + 1) * NT],
                            start=(m == 0),
                            stop=(m == KCH - 1),
                        )
            e2 = esb_p.tile([P, groups_of, NT], bf16, tag="e2", name="e2")
            nc.scalar.activation(
                out=e2.rearrange("p a b -> p (a b)"),
                in_=ps.rearrange("p a b -> p (a b)"),
                func=ACTF.Exp, scale=EXPS,
            )
            for j, (a, v, h) in enumerate(chunk):
                if a not in acc:
                    acc[a] = rsg_p.tile([P, NT], bf16, tag=f"acc{a}",
                                        name=f"acc{a}")
                    nc.vector.tensor_copy(acc[a], e2[:, j, :])
                else:
                    nc.vector.tensor_add(acc[a], acc[a], e2[:, j, :])
                if h == 0:
                    dscr = dscr_p.tile([P, P], bf16, tag="dscr", name="dscr")
                    nc.vector.scalar_tensor_tensor(
                        out=dscr, in0=e2[:, j, 0:P], scalar=1.0, in1=eye_sb,
                        op0=ALU.mult, op1=ALU.mult,
                        accum_out=dvals[:, a * NV + v : a * NV + v + 1],
                    )
            if g == 2:
                emit_dsq(0)
            elif g == 6:
                emit_dsq(1)

        # sqrt via exp(0.5*ln(x)): stays inside the ln+exp activation table.
        # cs rows are fp8 normalized*SCALE, so cs_term is SCALE*sum(norms);
        # the 1/SCALE rides the final 0.5 combine factor.
        lns = fin_p.tile([P, 2], fp32, tag="lns")
        nc.scalar.activation(out=lns, in_=sspack, func=ACTF.Ln)
        csreg = fin_p.tile([P, 2], fp32, tag="csreg")
        nc.scalar.activation(out=csreg, in_=lns, func=ACTF.Exp, scale=0.5)
        cs_term = fin_p.tile([P, 1], fp32, tag="cs_term")
        nc.vector.reduce_sum(out=cs_term, in_=csreg, axis=AX.X)

        # ---- final reduction ----
        rsums = fin_p.tile([P, NV], fp32, tag="rsums")
        for a in range(NV):
            nc.vector.reduce_sum(
                out=rsums[:, a : a + 1], in_=acc[a], axis=AX.X
            )
        s4s = fin_p.tile([P, NV], fp32, tag="s4s")
        nc.vector.reduce_sum(
            out=s4s, in_=dvals.rearrange("p (a b) -> p a b", a=NV), axis=AX.X
        )
        # self terms dvals[:, a*NV+a]: stride-(NV+1) diagonal view
        dd = bass.AP(
            tensor=dvals.tensor, offset=dvals.offset,
            ap=[dvals.ap[0], [NV + 1, NV]],
        )
        logpack = fin_p.tile([P, 2 * NV], fp32, tag="logpack")
        nc.vector.tensor_sub(logpack[:, 0:NV], rsums, dd)  # denom
        nc.vector.tensor_sub(logpack[:, NV : 2 * NV], s4s, dd)  # pos
        logs = fin_p.tile([P, 2 * NV], fp32, tag="logs")
        nc.scalar.activation(out=logs, in_=logpack, func=ACTF.Ln)
        s1 = fin_p.tile([P, 1], fp32, tag="s1")
        nc.vector.reduce_sum(out=s1, in_=logs[:, 0:NV], axis=AX.X)
        s2 = fin_p.tile([P, 1], fp32, tag="s2")
        nc.vector.reduce_sum(out=s2, in_=logs[:, NV : 2 * NV], axis=AX.X)
        contrib = fin_p.tile([P, 1], fp32, tag="contrib")
        nc.vector.tensor_sub(contrib, s1, s2)
        out_sb = fin_p.tile([P, 1], fp32, tag="out_sb")
        nc.vector.scalar_tensor_tensor(
            out=out_sb, in0=cs_term, scalar=0.5 / SCALE, in1=contrib,
            op0=ALU.mult, op1=ALU.add,
        )
        nc.sync.dma_start(out=out_d, in_=out_sb)

    nc.compile()
    return nc


def _get_nc():
    if "nc" not in _compiled:
        _compiled["nc"] = _build_kernel()
    return _compiled["nc"]


def _make_in_maps(english, etok, ktoe, korean, cs_ratios):
    e = np.asarray(english, dtype=np.float32)
    etk = np.asarray(etok, dtype=np.float32)
    kte = np.asarray(ktoe, dtype=np.float32)
    k = np.asarray(korean, dtype=np.float32)
    r = np.asarray(cs_ratios, dtype=np.float32)

    # version order must match the reference stack: [e, k, etk, kte]
    V4f = np.stack([e, k, etk, kte])  # [4, B, D] fp32
    V4n = V4f / np.linalg.norm(V4f, axis=2, keepdims=True)
    V4s = (V4n * SCALE).astype(ml_dtypes.float8_e4m3)
    eye = np.eye(P, dtype=ml_dtypes.bfloat16)

    in_maps = []
    for c in range(NC_CORES):
        rot = np.roll(V4s, -c * CHUNK, axis=1)  # [4, B, D], own chunk first
        embT = np.ascontiguousarray(rot.transpose(0, 2, 1)).reshape(NV * D, B)
        csrows = np.ascontiguousarray(rot[:, :P, :]).reshape(NV * P, D)
        rr = np.roll(r, -c * CHUNK)[:P].reshape(P, 1).astype(np.float32)
        in_maps.append(
            {"embT": embT, "csrows": csrows, "ratios": rr, "eye": eye}
        )
    return in_maps


def kernel(english, etok, ktoe, korean, cs_ratios):
    from concourse.bass_utils import run_bass_kernel_spmd

    in_maps = _make_in_maps(english, etok, ktoe, korean, cs_ratios)
    nc = _get_nc()
    res = run_bass_kernel_spmd(nc, in_maps, core_ids=list(range(NC_CORES)))
    total = 0.0
    for rmap in res.results:
        total += rmap["out"].astype(np.float64).sum()
    return np.array(total / B, dtype=np.float32)


# revision 24
# speedup vs baseline: 1.2987x; 1.2987x over previous
"""CodeSwitchLoss Trainium2 kernel (8-core data-parallel).

Math (see reference): V = l2norm rows of the stack [e, k, etk, kte] (4096 x 1024),
S = V @ V.T, E = exp(10*S).
Per anchor row r=(a,i):
  rowsum[r]   = sum_c E[r,c]
  d_b[r]      = E[r, col(b,i)]  (same-sample entries, b=0..3)
  pos[r]      = sum_{b != a} d_b[r]
  denom[r]    = rowsum[r] - d_a[r]          (= pos + neg)
  contrastive = log(denom) - log(pos)
plus cs regularization on normalized rows; total = (sum contrastive + 0.5*sum reg)/B.

Sharding: batch samples split 8 ways. Each core gets the full embedding set,
rolled so its own 128 samples come first; it computes the 512 anchor rows
(4 versions x 128 samples) against all 4096 columns. Scalar partials summed on
host. The roll makes all per-core slice offsets compile-time constants, so one
NEFF serves all 8 cores.

The host ships rows already l2-normalized (scaled by 32 so fp8e4m3 keeps its
relative precision) and pre-transposed to the matmul layout, so the device does
no norm computation at all: fp8 DoubleRow matmuls into PSUM, one exp per
4-bank group (scale folds in the 10/32^2 temperature factor), DVE rowsum
reduces + eye-masked diagonal extraction, and a short log/sqrt tail. The
measured diagonal is subtracted from both rowsum and pos, so the fp8 norm
noise on the huge self term cancels exactly.
"""

import numpy as np
import ml_dtypes

B = 1024
D = 1024
P = 128
NV = 4
NC_CORES = 8
CHUNK = B // NC_CORES  # 128 samples per core
KCH = D // P  # 8 k-chunks
NT = 512  # matmul free-dim tile (one PSUM bank)
SCALE = 32.0  # fp8 pre-scale on normalized rows
EXPS = 10.0 / (SCALE * SCALE)  # exp scale: 1/T divided by SCALE^2

_compiled = {}


def _sched():
    """Pair-half processing order matching W-version DMA arrival.

    Returns a list of (a, v, h): anchor version a (stationary, own 128
    samples), moving version v, column half h. (a,v,h) needs W[v] half h
    and W[a] cols 0:128 (inside h0)."""
    s = []
    for v in range(NV):
        for a in range(v + 1):
            s.append((a, v, 0))
        for b in range(v):
            s.append((v, b, 0))
            s.append((v, b, 1))
        for a in range(v + 1):
            s.append((a, v, 1))
    return s


def _build_kernel(warm=24, warm_free=128, groups_of=4, drow=True,
                  ps_bufs=2):
    from contextlib import ExitStack

    import concourse.bass as bass
    import concourse.tile as tile
    from concourse import bacc, mybir

    fp32 = mybir.dt.float32
    bf16 = mybir.dt.bfloat16
    fp8 = mybir.dt.float8e4
    AX = mybir.AxisListType
    ALU = mybir.AluOpType
    ACTF = mybir.ActivationFunctionType

    nc = bacc.Bacc(
        "TRN2",
        target_bir_lowering=False,
        debug=False,
        enable_asserts=False,
        num_devices=NC_CORES,
    )
    # pre-transposed normalized*32 fp8 embeddings: embT[v*D + d, s] = W_v[s, d]
    embT = nc.dram_tensor("embT", [NV * D, B], fp8, kind="ExternalInput").ap()
    # natural-layout normalized*32 rows of this core's own chunk (for cs reg)
    csrows = nc.dram_tensor("csrows", [NV * P, D], fp8, kind="ExternalInput").ap()
    ratios = nc.dram_tensor("ratios", [P, 1], fp32, kind="ExternalInput").ap()
    eye_d = nc.dram_tensor("eye", [P, P], bf16, kind="ExternalInput").ap()
    out_d = nc.dram_tensor("out", [P, 1], fp32, kind="ExternalOutput").ap()

    sched = _sched()
    ngroups = len(sched) // groups_of
    # h0 slot index per pair (for diag extraction)
    h0_slot = {}
    for slot, (a, v, h) in enumerate(sched):
        if h == 0:
            h0_slot[(a, v)] = slot

    with tile.TileContext(nc) as tc, ExitStack() as ctx:
        consts = ctx.enter_context(tc.tile_pool(name="consts", bufs=1))
        wpool = ctx.enter_context(tc.tile_pool(name="w", bufs=1))
        psum_p = ctx.enter_context(
            tc.tile_pool(name="psum", bufs=ps_bufs, space="PSUM")
        )
        esb_p = ctx.enter_context(tc.tile_pool(name="esb", bufs=3))
        csx_p = ctx.enter_context(tc.tile_pool(name="csx", bufs=1))
        scr_p = ctx.enter_context(tc.tile_pool(name="scr", bufs=2))
        dscr_p = ctx.enter_context(tc.tile_pool(name="dscr", bufs=3))
        rsg_p = ctx.enter_context(tc.tile_pool(name="rsg", bufs=1))
        fin_p = ctx.enter_context(tc.tile_pool(name="fin", bufs=1))

        # Pre-load the activation table set holding BOTH Exp and Ln, so the
        # compiler's table-load pass never has to insert a (1.3us) reload
        # between the exp stream and the log/sqrt tail.
        from concourse.hw_specs import get_activation_tables

        tabs = list(get_activation_tables(nc.m.arch).values())
        set_id = next(
            i for i, s in enumerate(tabs)
            if ACTF.Exp in s and ACTF.Ln in s
        )
        nc.scalar.add_instruction(
            mybir.InstLoadActFuncSet(
                name=nc.scalar.bass.get_next_instruction_name(),
                ins=[], outs=[], act_func_set_id=set_id,
            )
        )

        # --- constants / warmup (PE busy from t=0 so the clock is ramped
        # to max by the time the first real matmul's W tile has landed) ---
        ones_sb = consts.tile([P, warm_free], bf16, tag="ones")
        nc.vector.memset(ones_sb, 1.0)
        ones1 = consts.tile([P, 1], fp32, tag="ones1")
        nc.vector.memset(ones1, 1.0)
        eye_sb = consts.tile([P, P], bf16, tag="eye")
        nc.gpsimd.dma_start(out=eye_sb, in_=eye_d)
        r_sb = consts.tile([P, 1], fp32, tag="ratios")
        nc.gpsimd.dma_start(out=r_sb, in_=ratios)

        W = [
            wpool.tile([P, KCH, B], fp8, tag=f"w{v}", name=f"w{v}")
            for v in range(NV)
        ]

        # --- input DMAs (SP/HWDGE, in arrival order the schedule expects).
        # csrows goes first: it is small (fp8) and unblocks the cs-reg chain,
        # which then runs on DVE/Pool during their otherwise-idle prologue.
        # W3's stationary slice (cols 0:128) is pulled forward so (3,b,*)
        # pairs unlock as soon as their moving version is resident.
        csx = csx_p.tile([P, NV, D], fp8, tag="csx")
        nc.sync.dma_start(
            out=csx, in_=csrows.rearrange("(v p) d -> p v d", p=P)
        )

        def w_dma(v, c0, c1):
            nc.sync.dma_start(
                out=W[v][:, :, c0:c1],
                in_=embT[v * D : (v + 1) * D, c0:c1].rearrange(
                    "(mm p) s -> p mm s", p=P
                ),
            )

        w_dma(0, 0, NT)
        w_dma(3, 0, P)
        w_dma(0, NT, B)
        w_dma(1, 0, NT)
        w_dma(1, NT, B)
        w_dma(2, 0, NT)
        w_dma(2, NT, B)
        w_dma(3, P, NT)
        w_dma(3, NT, B)

        # warmup matmuls: no data deps beyond the memset; overwritten later
        warm_ps = psum_p.tile([P, groups_of, NT], fp32, tag="ps", name="ps_w")
        for i in range(warm):
            nc.tensor.matmul(
                warm_ps[:, i % groups_of, 0:warm_free],
                ones_sb, ones_sb, start=True, stop=True,
            )

        # ---- cs regularization on own chunk (emitted up-front so it runs
        # in the DVE/Pool prologue idle window; branch 1 on DVE, branch 2 on
        # Pool). Rows are normalized*32 fp8; the /32 is folded into the
        # sqrt's exp bias below. ----
        e0, k0, etk0, kte0 = (csx[:, vv, :] for vv in range(NV))
        sspack = fin_p.tile([P, 2], fp32, tag="sspack")
        t1 = scr_p.tile([P, D], bf16, tag="cs_t")
        nc.gpsimd.tensor_sub(t1, e0, k0)
        u = scr_p.tile([P, D], bf16, tag="cs_u")
        nc.vector.tensor_scalar_mul(u, t1, r_sb)
        d1 = scr_p.tile([P, D], bf16, tag="cs_t")
        nc.gpsimd.tensor_sub(d1, etk0, k0)
        d1m = scr_p.tile([P, D], bf16, tag="cs_d")
        nc.gpsimd.tensor_sub(d1m, d1, u)
        d2 = scr_p.tile([P, D], bf16, tag="cs_t2")
        nc.gpsimd.tensor_sub(d2, kte0, e0)
        d2m = scr_p.tile([P, D], bf16, tag="cs_d2")
        nc.gpsimd.tensor_add(d2m, d2, u)

        def emit_dsq(which, dval_idx):
            # The Tile scheduler statically orders each engine's queue by its
            # own readiness simulation (which models DMAs as instant), so
            # without a real dependency these would land ahead of main-loop
            # DVE work and stall it at runtime. The gate (==1.0, derived from
            # a diag value of the current group) pins them to mid-loop.
            src = d1m if which == 0 else d2m
            gate = fin_p.tile([P, 1], fp32, tag=f"gate{which}")
            nc.vector.scalar_tensor_tensor(
                out=gate, in0=dvals[:, dval_idx : dval_idx + 1],
                scalar=0.0, in1=ones1, op0=ALU.mult, op1=ALU.add,
            )
            dsq = scr_p.tile([P, D], bf16, tag=f"cs_q{which}")
            nc.vector.scalar_tensor_tensor(
                out=dsq, in0=src, scalar=gate, in1=src,
                op0=ALU.mult, op1=ALU.mult,
                accum_out=sspack[:, which : which + 1],
            )

        # --- main loop: groups of `groups_of` pair-halves ---
        # rowsum partials accumulate per-anchor in bf16 via TensorTensor
        # adds (the only DVE op class with a fast mode); diag extraction
        # runs on the otherwise-idle Pool engine.
        acc = {}
        dvals = fin_p.tile([P, NV * NV], fp32, tag="dvals")  # [:, a*NV + v]

        for g in range(ngroups):
            chunk = sched[g * groups_of : (g + 1) * groups_of]
            ps = psum_p.tile([P, groups_of, NT], fp32, tag="ps", name="ps")
            for j, (a, v, h) in enumerate(chunk):
                if drow:
                    for m in range(0, KCH, 2):
                        nc.tensor.matmul(
                            ps[:, j, :],
                            W[a][:, m : m + 2, 0:P],
                            W[v][:, m : m + 2, h * NT : (h + 1) * NT],
                            start=(m == 0),
                            stop=(m == KCH - 2),
                            perf_mode=mybir.MatmulPerfMode.DoubleRow,
                        )
                else:
                    for m in range(KCH):
                        nc.tensor.matmul(
                            ps[:, j, :],
                            W[a][:, m, 0:P],
                            W[v][:, m, h * NT : (h + 1) * NT],
                            start=(m == 0),
                            stop=(m == KCH - 1),
                        )
            e2 = esb_p.tile([P, groups_of, NT], bf16, tag="e2", name="e2")
            nc.scalar.activation(
                out=e2.rearrange("p a b -> p (a b)"),
                in_=ps.rearrange("p a b -> p (a b)"),
                func=ACTF.Exp, scale=EXPS,
            )
            for j, (a, v, h) in enumerate(chunk):
                if a not in acc:
                    acc[a] = rsg_p.tile([P, NT], bf16, tag=f"acc{a}",
                                        name=f"acc{a}")
                    nc.vector.tensor_copy(acc[a], e2[:, j, :])
                else:
                    nc.vector.tensor_add(acc[a], acc[a], e2[:, j, :])
                if h == 0:
                    dscr = dscr_p.tile([P, P], bf16, tag="dscr", name="dscr")
                    nc.vector.scalar_tensor_tensor(
                        out=dscr, in0=e2[:, j, 0:P], scalar=1.0, in1=eye_sb,
                        op0=ALU.mult, op1=ALU.mult,
                        accum_out=dvals[:, a * NV + v : a * NV + v + 1],
                    )
            if g in (2, 6):
                last_diag = next(
                    aa * NV + vv for aa, vv, hh in reversed(chunk) if hh == 0
                )
                emit_dsq(0 if g == 2 else 1, last_diag)

        # sqrt via exp(0.5*ln(x)): stays inside the ln+exp activation table.
        # cs rows are fp8 normalized*SCALE, so cs_term is SCALE*sum(norms);
        # the 1/SCALE rides the final 0.5 combine factor.
        lns = fin_p.tile([P, 2], fp32, tag="lns")
        nc.scalar.activation(out=lns, in_=sspack, func=ACTF.Ln)
        csreg = fin_p.tile([P, 2], fp32, tag="csreg")
        nc.scalar.activation(out=csreg, in_=lns, func=ACTF.Exp, scale=0.5)
        cs_term = fin_p.tile([P, 1], fp32, tag="cs_term")
        nc.vector.reduce_sum(out=cs_term, in_=csreg, axis=AX.X)

        # ---- final reduction ----
        rsums = fin_p.tile([P, NV], fp32, tag="rsums")
        for a in range(NV):
            nc.vector.reduce_sum(
                out=rsums[:, a : a + 1], in_=acc[a], axis=AX.X
            )
        s4s = fin_p.tile([P, NV], fp32, tag="s4s")
        nc.vector.reduce_sum(
            out=s4s, in_=dvals.rearrange("p (a b) -> p a b", a=NV), axis=AX.X
        )
        # self terms dvals[:, a*NV+a]: stride-(NV+1) diagonal view
        dd = bass.AP(
            tensor=dvals.tensor, offset=dvals.offset,
            ap=[dvals.ap[0], [NV + 1, NV]],
        )
        logpack = fin_p.tile([P, 2 * NV], fp32, tag="logpack")
        nc.vector.tensor_sub(logpack[:, 0:NV], rsums, dd)  # denom
        nc.vector.tensor_sub(logpack[:, NV : 2 * NV], s4s, dd)  # pos
        logs = fin_p.tile([P, 2 * NV], fp32, tag="logs")
        nc.scalar.activation(out=logs, in_=logpack, func=ACTF.Ln)
        s1 = fin_p.tile([P, 1], fp32, tag="s1")
        nc.vector.reduce_sum(out=s1, in_=logs[:, 0:NV], axis=AX.X)
        s2 = fin_p.tile([P, 1], fp32, tag="s2")
        nc.vector.reduce_sum(out=s2, in_=logs[:, NV : 2 * NV], axis=AX.X)
        contrib = fin_p.tile([P, 1], fp32, tag="contrib")
        nc.vector.tensor_sub(contrib, s1, s2)
        out_sb = fin_p.tile([P, 1], fp32, tag="out_sb")
        nc.vector.scalar_tensor_tensor(
            out=out_sb, in0=cs_term, scalar=0.5 / SCALE, in1=contrib,
            op0=ALU.mult, op1=ALU.add,
        )
        nc.sync.dma_start(out=out_d, in_=out_sb)

    nc.compile()
    return nc


def _get_nc():
    if "nc" not in _compiled:
        _compiled["nc"] = _build_kernel()
    return _compiled["nc"]


def _make_in_maps(english, etok, ktoe, korean, cs_ratios):
    e = np.asarray(english, dtype=np.float32)
    etk = np.asarray(etok, dtype=np.float32)
    kte = np.asarray(ktoe, dtype=np.float32)
    k = np.asarray(korean, dtype=np.float32)
    r = np.asarray(cs_ratios, dtype=np.float32)

    # version order must match the reference stack: [e, k, etk, kte]
    V4f = np.stack([e, k, etk, kte])  # [4, B, D] fp32
    V4n = V4f / np.linalg.norm(V4f, axis=2, keepdims=True)
    V4s = (V4n * SCALE).astype(ml_dtypes.float8_e4m3)
    eye = np.eye(P, dtype=ml_dtypes.bfloat16)

    in_maps = []
    for c in range(NC_CORES):
        rot = np.roll(V4s, -c * CHUNK, axis=1)  # [4, B, D], own chunk first
        embT = np.ascontiguousarray(rot.transpose(0, 2, 1)).reshape(NV * D, B)
        csrows = np.ascontiguousarray(rot[:, :P, :]).reshape(NV * P, D)
        rr = np.roll(r, -c * CHUNK)[:P].reshape(P, 1).astype(np.float32)
        in_maps.append(
            {"embT": embT, "csrows": csrows, "ratios": rr, "eye": eye}
        )
    return in_maps


def kernel(english, etok, ktoe, korean, cs_ratios):
    from concourse.bass_utils import run_bass_kernel_spmd

    in_maps = _make_in_maps(english, etok, ktoe, korean, cs_ratios)
    nc = _get_nc()
    res = run_bass_kernel_spmd(nc, in_maps, core_ids=list(range(NC_CORES)))
    total = 0.0
    for rmap in res.results:
        total += rmap["out"].astype(np.float64).sum()
    return np.array(total / B, dtype=np.float32)


# revision 39
# speedup vs baseline: 1.4308x; 1.1017x over previous
"""CodeSwitchLoss Trainium2 kernel (8-core data-parallel).

Math (see reference): V = l2norm rows of the stack [e, k, etk, kte] (4096 x 1024),
S = V @ V.T, E = exp(10*S).
Per anchor row r=(a,i):
  rowsum[r]   = sum_c E[r,c]
  d_b[r]      = E[r, col(b,i)]  (same-sample entries, b=0..3)
  pos[r]      = sum_{b != a} d_b[r]
  denom[r]    = rowsum[r] - d_a[r]          (= pos + neg)
  contrastive = log(denom) - log(pos)
plus cs regularization on normalized rows; total = (sum contrastive + 0.5*sum reg)/B.

Sharding: batch samples split 8 ways. Each core gets the full embedding set,
rolled so its own 128 samples come first; it computes the 512 anchor rows
(4 versions x 128 samples) against all 4096 columns. Scalar partials summed on
host. The roll makes all per-core slice offsets compile-time constants, so one
NEFF serves all 8 cores.

The host ships rows already l2-normalized (scaled by 32 so fp8e4m3 keeps its
relative precision) and pre-transposed to the matmul layout, so the device does
no norm computation at all: fp8 DoubleRow matmuls into PSUM, one exp per
4-bank group (scale folds in the 10/32^2 temperature factor), DVE rowsum
reduces + eye-masked diagonal extraction, and a short log/sqrt tail. The
measured diagonal is subtracted from both rowsum and pos, so the fp8 norm
noise on the huge self term cancels exactly.
"""

import numpy as np
import ml_dtypes

B = 1024
D = 1024
P = 128
NV = 4
NC_CORES = 8
CHUNK = B // NC_CORES  # 128 samples per core
KCH = D // P  # 8 k-chunks
NT = 512  # matmul free-dim tile (one PSUM bank)
SCALE = 32.0  # fp8 pre-scale on normalized rows
EXPS = 10.0 / (SCALE * SCALE)  # exp scale: 1/T divided by SCALE^2

_compiled = {}


def _sched():
    """Pair-half processing order matching W-version DMA arrival.

    Returns a list of (a, v, h): anchor version a (stationary, own 128
    samples), moving version v, column half h. (a,v,h) needs W[v] half h
    and W[a] cols 0:128 (inside h0)."""
    s = []
    for v in range(NV):
        for a in range(v + 1):
            s.append((a, v, 0))
        for b in range(v):
            s.append((v, b, 0))
            s.append((v, b, 1))
        for a in range(v + 1):
            s.append((a, v, 1))
    return s


def _build_kernel(warm=24, warm_free=128, groups_of=4, drow=True,
                  ps_bufs=2):
    from contextlib import ExitStack

    import concourse.bass as bass
    import concourse.tile as tile
    from concourse import bacc, mybir

    fp32 = mybir.dt.float32
    bf16 = mybir.dt.bfloat16
    fp8 = mybir.dt.float8e4
    AX = mybir.AxisListType
    ALU = mybir.AluOpType
    ACTF = mybir.ActivationFunctionType

    nc = bacc.Bacc(
        "TRN2",
        target_bir_lowering=False,
        debug=False,
        enable_asserts=False,
        num_devices=NC_CORES,
    )
    # pre-transposed normalized*32 fp8 embeddings: embT[v*D + d, s] = W_v[s, d]
    embT = nc.dram_tensor("embT", [NV * D, B], fp8, kind="ExternalInput").ap()
    # natural-layout normalized*32 rows of this core's own chunk (for cs reg)
    csrows = nc.dram_tensor("csrows", [NV * P, D], fp8, kind="ExternalInput").ap()
    ratios = nc.dram_tensor("ratios", [P, 1], fp32, kind="ExternalInput").ap()
    eye_d = nc.dram_tensor("eye", [P, P], bf16, kind="ExternalInput").ap()
    out_d = nc.dram_tensor("out", [P, 1], fp32, kind="ExternalOutput").ap()

    sched = _sched()
    ngroups = len(sched) // groups_of
    # h0 slot index per pair (for diag extraction)
    h0_slot = {}
    for slot, (a, v, h) in enumerate(sched):
        if h == 0:
            h0_slot[(a, v)] = slot

    with tile.TileContext(nc) as tc, ExitStack() as ctx:
        consts = ctx.enter_context(tc.tile_pool(name="consts", bufs=1))
        wpool = ctx.enter_context(tc.tile_pool(name="w", bufs=1))
        psum_p = ctx.enter_context(
            tc.tile_pool(name="psum", bufs=ps_bufs, space="PSUM")
        )
        lead_pool = ctx.enter_context(
            tc.tile_pool(name="leadps", bufs=1, space="PSUM")
        )
        esb_p = ctx.enter_context(tc.tile_pool(name="esb", bufs=3))
        csx_p = ctx.enter_context(tc.tile_pool(name="csx", bufs=1))
        scr_p = ctx.enter_context(tc.tile_pool(name="scr", bufs=2))
        dscr_p = ctx.enter_context(tc.tile_pool(name="dscr", bufs=3))
        rsg_p = ctx.enter_context(tc.tile_pool(name="rsg", bufs=1))
        fin_p = ctx.enter_context(tc.tile_pool(name="fin", bufs=1))

        # Pre-load the activation table set holding BOTH Exp and Ln, so the
        # compiler's table-load pass never has to insert a (1.3us) reload
        # between the exp stream and the log/sqrt tail.
        from concourse.hw_specs import get_activation_tables

        tabs = list(get_activation_tables(nc.m.arch).values())
        set_id = next(
            i for i, s in enumerate(tabs)
            if ACTF.Exp in s and ACTF.Ln in s
        )
        nc.scalar.add_instruction(
            mybir.InstLoadActFuncSet(
                name=nc.scalar.bass.get_next_instruction_name(),
                ins=[], outs=[], act_func_set_id=set_id,
            )
        )

        # --- constants / warmup (PE busy from t=0 so the clock is ramped
        # to max by the time the first real matmul's W tile has landed) ---
        ones_sb = consts.tile([P, warm_free], bf16, tag="ones")
        nc.vector.memset(ones_sb, 1.0)
        ones1 = consts.tile([P, 1], fp32, tag="ones1")
        nc.vector.memset(ones1, 1.0)
        eye_sb = consts.tile([P, P], bf16, tag="eye")
        nc.gpsimd.dma_start(out=eye_sb, in_=eye_d)
        r_sb = consts.tile([P, 1], fp32, tag="ratios")
        nc.gpsimd.dma_start(out=r_sb, in_=ratios)

        W = [
            wpool.tile([P, KCH, B], fp8, tag=f"w{v}", name=f"w{v}")
            for v in range(NV)
        ]

        # --- input DMAs (SP/HWDGE, in arrival order the schedule expects).
        # csrows goes first: it is small (fp8) and unblocks the cs-reg chain,
        # which then runs on DVE/Pool during their otherwise-idle prologue.
        # W3's stationary slice (cols 0:128) is pulled forward so (3,b,*)
        # pairs unlock as soon as their moving version is resident.
        def w_dma(v, c0, c1):
            nc.sync.dma_start(
                out=W[v][:, :, c0:c1],
                in_=embT[v * D : (v + 1) * D, c0:c1].rearrange(
                    "(mm p) s -> p mm s", p=P
                ),
            )

        csx = csx_p.tile([P, NV, D], fp8, tag="csx")
        nc.sync.dma_start(
            out=csx[:, 0:2, :],
            in_=csrows[0 : 2 * P, :].rearrange("(v p) d -> p v d", p=P),
        )
        w_dma(0, 0, NT)
        nc.sync.dma_start(
            out=csx[:, 2:4, :],
            in_=csrows[2 * P : 4 * P, :].rearrange("(v p) d -> p v d", p=P),
        )
        w_dma(0, NT, B)
        w_dma(1, 0, NT)
        w_dma(1, NT, B)
        w_dma(2, 0, NT)
        w_dma(2, NT, B)
        w_dma(3, 0, NT)
        w_dma(3, NT, B)

        # ---- cs regularization on own chunk: the whole diff chain runs
        # on the otherwise-idle Pool engine during the prologue (u uses a
        # stride-0 broadcast of the ratio column, since tensor-scalar ops
        # are not legal on Pool). The squares+row-sums run on Act as
        # Square-activations with accumulate, bias-gated on the last
        # group's exp output so the scheduler cannot order them ahead of
        # the exp stream. Rows are normalized*SCALE fp8; the 1/SCALE rides
        # the final combine. ----
        e0, k0, etk0, kte0 = (csx[:, vv, :] for vv in range(NV))
        sspack = fin_p.tile([P, 2], fp32, tag="sspack")
        t1 = scr_p.tile([P, D], bf16, tag="cs_t")
        nc.gpsimd.tensor_sub(t1, e0, k0)
        r_bcast = bass.AP(
            tensor=r_sb.tensor, offset=r_sb.offset,
            ap=[r_sb.ap[0], [0, D]],
        )
        u = scr_p.tile([P, D], bf16, tag="cs_u")
        nc.gpsimd.tensor_mul(u, t1, r_bcast)
        d1 = scr_p.tile([P, D], bf16, tag="cs_t")
        nc.gpsimd.tensor_sub(d1, etk0, k0)
        d1m = scr_p.tile([P, D], bf16, tag="cs_d")
        nc.gpsimd.tensor_sub(d1m, d1, u)
        d2 = scr_p.tile([P, D], bf16, tag="cs_t2")
        nc.gpsimd.tensor_sub(d2, kte0, e0)
        d2m = scr_p.tile([P, D], bf16, tag="cs_d2")
        nc.gpsimd.tensor_add(d2m, d2, u)
        # branch-2 square + fold chain entirely on Pool (TT ops are legal
        # there); folded to [P,8], summed into sspack by a tiny gated TTR
        qm2 = scr_p.tile([P, D], bf16, tag="cs_qم2" if False else "cs_qm2")
        nc.gpsimd.tensor_mul(qm2, d2m, d2m)
        qf_prev, width = qm2, D
        while width > 8:
            width //= 2
            qf = scr_p.tile([P, width], bf16, tag=f"cs_qf{width}")
            nc.gpsimd.tensor_add(
                qf, qf_prev[:, 0:width], qf_prev[:, width : 2 * width]
            )
            qf_prev = qf
        qf8 = qf_prev

        dvals = fin_p.tile([P, NV * NV], fp32, tag="dvals")  # [:, a*NV + v]

        # --- main loop over pair-half groups. Fine-grained lead-in groups
        # start the exp stream as soon as W0h0 lands; main groups are
        # 2-half (2 PSUM banks, double-buffered) so PSUM recycles quickly;
        # the four W3h1-gated tail halves write a dedicated tile that
        # reuses the lead tile's banks (free since ~t=11us), so their
        # matmuls run the moment W3h1 arrives, and their exps accumulate
        # their rowsum contributions directly (no DVE tail reduces). ---
        groups = (
            [sched[0:1], sched[1:2], sched[2:4]]
            + [sched[4 + 2 * i : 6 + 2 * i] for i in range(12)]
            + [sched[28 + i : 29 + i] for i in range(4)]
        )
        n_tail = 4
        acc = {}
        acc_last_group = {}
        for gi, chunk in enumerate(groups[: len(groups) - n_tail]):
            for a, v, h in chunk:
                acc_last_group[a] = gi
        rsums_pre = fin_p.tile([P, NV], fp32, tag="rsums_pre")
        rsums_d = fin_p.tile([P, NV], fp32, tag="rsums_d")
        s4s = fin_p.tile([P, NV], fp32, tag="s4s")
        logpack = fin_p.tile([P, 2 * NV], fp32, tag="logpack")
        # self terms dvals[:, a*NV+a]: stride-(NV+1) diagonal view
        dd = bass.AP(
            tensor=dvals.tensor, offset=dvals.offset,
            ap=[dvals.ap[0], [NV + 1, NV]],
        )
        last_h0_group = max(
            gi for gi, chunk in enumerate(groups)
            if any(hh == 0 for _, _, hh in chunk)
        )

        def mk_gate(tag, dval_idx):
            # ==1.0 tile whose data dep pins downstream ops to a known spot
            # in the DVE static order (the Tile scheduler statically orders
            # each engine queue by its own readiness simulation, which
            # models DMAs as instant — ungated, the cs ops would land ahead
            # of main-loop DVE work in the static order and stall it)
            gate = fin_p.tile([P, 1], fp32, tag=tag)
            nc.vector.scalar_tensor_tensor(
                out=gate, in0=dvals[:, dval_idx : dval_idx + 1],
                scalar=0.0, in1=ones1, op0=ALU.mult, op1=ALU.add,
            )
            return gate

        lead_ps = lead_pool.tile([P, 4, NT], fp32, tag="lead", name="ps_lead")
        first_tail = len(groups) - n_tail
        tail_ps = None
        for i in range(warm):
            nc.tensor.matmul(
                lead_ps[:, i % 4, 0:warm_free],
                ones_sb, ones_sb, start=True, stop=True,
            )
        for g, chunk in enumerate(groups):
            gw = len(chunk)
            tail = g >= first_tail
            if g == 0:
                ps = lead_ps[:, 0:1, :]
            elif g == 1:
                ps = lead_ps[:, 1:2, :]
            elif g == 2:
                ps = lead_ps[:, 2:4, :]
            elif tail:
                # the tail tile reuses the lead banks (free after the lead-in
                # exps), so these matmuls run the moment W3h1 arrives
                if tail_ps is None:
                    tail_ps = lead_pool.tile([P, 4, NT], fp32, tag="lead",
                                             name="ps_tail")
                ti = g - first_tail
                ps = tail_ps[:, ti : ti + 1, :]
            else:
                ps = psum_p.tile([P, 2, NT], fp32, tag="ps", name="ps")
            for j, (a, v, h) in enumerate(chunk):
                for m in range(0, KCH, 2):
                    nc.tensor.matmul(
                        ps[:, j, :],
                        W[a][:, m : m + 2, 0:P],
                        W[v][:, m : m + 2, h * NT : (h + 1) * NT],
                        start=(m == 0),
                        stop=(m == KCH - 2),
                        perf_mode=mybir.MatmulPerfMode.DoubleRow,
                    )
            e2 = esb_p.tile([P, gw, NT], bf16, tag=f"e2w{gw}", name="e2")
            if tail:
                a = chunk[0][0]
                nc.scalar.activation(
                    out=e2[:, 0, :], in_=ps[:, 0, :],
                    func=ACTF.Exp, scale=EXPS,
                    accum_out=rsums_d[:, a : a + 1],
                )
                continue
            nc.scalar.activation(
                out=e2.rearrange("p a b -> p (a b)"),
                in_=ps[:, 0:gw, :].rearrange("p a b -> p (a b)"),
                func=ACTF.Exp, scale=EXPS,
            )
            for j, (a, v, h) in enumerate(chunk):
                if a not in acc:
                    acc[a] = rsg_p.tile([P, NT], bf16, tag=f"acc{a}",
                                        name=f"acc{a}")
                    nc.vector.tensor_copy(acc[a], e2[:, j, :])
                else:
                    nc.vector.tensor_add(acc[a], acc[a], e2[:, j, :])
                if h == 0:
                    dscr = dscr_p.tile([P, P], bf16, tag="dscr", name="dscr")
                    nc.vector.scalar_tensor_tensor(
                        out=dscr, in0=e2[:, j, 0:P], scalar=1.0, in1=eye_sb,
                        op0=ALU.mult, op1=ALU.mult,
                        accum_out=dvals[:, a * NV + v : a * NV + v + 1],
                    )
            # per-anchor rowsum partials finish as soon as the anchor's
            # last accumulated half lands — overlaps the remaining groups
            for a in range(NV):
                if acc_last_group.get(a) == g:
                    nc.vector.reduce_sum(
                        out=rsums_pre[:, a : a + 1], in_=acc[a], axis=AX.X
                    )
            h0s = [aa * NV + vv for aa, vv, hh in chunk if hh == 0]
            if g == 7 and h0s:
                # cs branch-1 square: gated to this group's position
                gate5 = mk_gate("gate5", h0s[-1])
                dsq1 = scr_p.tile([P, D], bf16, tag="cs_q1")
                nc.vector.scalar_tensor_tensor(
                    out=dsq1, in0=d1m, scalar=gate5, in1=d1m,
                    op0=ALU.mult, op1=ALU.mult,
                    accum_out=sspack[:, 0:1],
                )
            elif g == 11 and h0s:
                # cs branch-2: sum the Pool-folded squares (tiny gated TTR)
                gate8 = mk_gate("gate8", h0s[-1])
                qdump = fin_p.tile([P, 8], bf16, tag="qdump")
                nc.vector.scalar_tensor_tensor(
                    out=qdump, in0=qf8, scalar=gate8, in1=ones_sb[:, 0:8],
                    op0=ALU.mult, op1=ALU.mult,
                    accum_out=sspack[:, 1:2],
                )
            elif g == last_h0_group:
                # dvals is complete after this group: fold the positives
                nc.vector.reduce_sum(
                    out=s4s,
                    in_=dvals.rearrange("p (a b) -> p a b", a=NV),
                    axis=AX.X,
                )
                nc.vector.tensor_sub(logpack[:, NV : 2 * NV], s4s, dd)  # pos

        # sqrt via exp(0.5*ln(x)): stays inside the ln+exp activation table
        lns = fin_p.tile([P, 2], fp32, tag="lns")
        nc.scalar.activation(out=lns, in_=sspack, func=ACTF.Ln)
        csreg = fin_p.tile([P, 2], fp32, tag="csreg")
        nc.scalar.activation(out=csreg, in_=lns, func=ACTF.Exp, scale=0.5)
        cs_term = fin_p.tile([P, 1], fp32, tag="cs_term")
        nc.vector.reduce_sum(out=cs_term, in_=csreg, axis=AX.X)

        # ---- final reduction ----
        rsums = fin_p.tile([P, NV], fp32, tag="rsums")
        nc.vector.tensor_add(rsums, rsums_pre, rsums_d)
        nc.vector.tensor_sub(logpack[:, 0:NV], rsums, dd)  # denom
        logs = fin_p.tile([P, 2 * NV], fp32, tag="logs")
        nc.scalar.activation(out=logs, in_=logpack, func=ACTF.Ln)
        s1 = fin_p.tile([P, 1], fp32, tag="s1")
        nc.vector.reduce_sum(out=s1, in_=logs[:, 0:NV], axis=AX.X)
        s2 = fin_p.tile([P, 1], fp32, tag="s2")
        nc.vector.reduce_sum(out=s2, in_=logs[:, NV : 2 * NV], axis=AX.X)
        contrib = fin_p.tile([P, 1], fp32, tag="contrib")
        nc.vector.tensor_sub(contrib, s1, s2)
        out_sb = fin_p.tile([P, 1], fp32, tag="out_sb")
        nc.vector.scalar_tensor_tensor(
            out=out_sb, in0=cs_term, scalar=0.5 / SCALE, in1=contrib,
            op0=ALU.mult, op1=ALU.add,
        )
        nc.sync.dma_start(out=out_d, in_=out_sb)

    nc.compile()
    return nc


def _get_nc():
    if "nc" not in _compiled:
        _compiled["nc"] = _build_kernel()
    return _compiled["nc"]


def _make_in_maps(english, etok, ktoe, korean, cs_ratios):
    e = np.asarray(english, dtype=np.float32)
    etk = np.asarray(etok, dtype=np.float32)
    kte = np.asarray(ktoe, dtype=np.float32)
    k = np.asarray(korean, dtype=np.float32)
    r = np.asarray(cs_ratios, dtype=np.float32)

    # version order must match the reference stack: [e, k, etk, kte]
    V4f = np.stack([e, k, etk, kte])  # [4, B, D] fp32
    V4n = V4f / np.linalg.norm(V4f, axis=2, keepdims=True)
    V4s = (V4n * SCALE).astype(ml_dtypes.float8_e4m3)
    eye = np.eye(P, dtype=ml_dtypes.bfloat16)

    in_maps = []
    for c in range(NC_CORES):
        rot = np.roll(V4s, -c * CHUNK, axis=1)  # [4, B, D], own chunk first
        embT = np.ascontiguousarray(rot.transpose(0, 2, 1)).reshape(NV * D, B)
        csrows = np.ascontiguousarray(rot[:, :P, :]).reshape(NV * P, D)
        rr = np.roll(r, -c * CHUNK)[:P].reshape(P, 1).astype(np.float32)
        in_maps.append(
            {"embT": embT, "csrows": csrows, "ratios": rr, "eye": eye}
        )
    return in_maps


def kernel(english, etok, ktoe, korean, cs_ratios):
    from concourse.bass_utils import run_bass_kernel_spmd

    in_maps = _make_in_maps(english, etok, ktoe, korean, cs_ratios)
    nc = _get_nc()
    res = run_bass_kernel_spmd(nc, in_maps, core_ids=list(range(NC_CORES)))
    total = 0.0
    for rmap in res.results:
        total += rmap["out"].astype(np.float64).sum()
    return np.array(total / B, dtype=np.float32)


# revision 45
# speedup vs baseline: 1.4946x; 1.0446x over previous
"""CodeSwitchLoss Trainium2 kernel (8-core data-parallel).

Math (see reference): V = l2norm rows of the stack [e, k, etk, kte] (4096 x 1024),
S = V @ V.T, E = exp(10*S).
Per anchor row r=(a,i):
  rowsum[r]   = sum_c E[r,c]
  d_b[r]      = E[r, col(b,i)]  (same-sample entries, b=0..3)
  pos[r]      = sum_{b != a} d_b[r]
  denom[r]    = rowsum[r] - d_a[r]          (= pos + neg)
  contrastive = log(denom) - log(pos)
plus cs regularization on normalized rows; total = (sum contrastive + 0.5*sum reg)/B.

Sharding: batch samples split 8 ways. Each core gets the full embedding set,
rolled so its own 128 samples come first; it computes the 512 anchor rows
(4 versions x 128 samples) against all 4096 columns. Scalar partials summed on
host. The roll makes all per-core slice offsets compile-time constants, so one
NEFF serves all 8 cores.

The host ships rows already l2-normalized (scaled by 32 so fp8e4m3 keeps its
relative precision) and pre-transposed to the matmul layout, so the device does
no norm computation at all: fp8 DoubleRow matmuls into PSUM, one exp per
4-bank group (scale folds in the 10/32^2 temperature factor), DVE rowsum
reduces + eye-masked diagonal extraction, and a short log/sqrt tail. The
measured diagonal is subtracted from both rowsum and pos, so the fp8 norm
noise on the huge self term cancels exactly.
"""

import numpy as np
import ml_dtypes

B = 1024
D = 1024
P = 128
NV = 4
NC_CORES = 8
CHUNK = B // NC_CORES  # 128 samples per core
KCH = D // P  # 8 k-chunks
NT = 512  # matmul free-dim tile (one PSUM bank)
SCALE = 32.0  # fp8 pre-scale on normalized rows
EXPS = 10.0 / (SCALE * SCALE)  # exp scale: 1/T divided by SCALE^2

_compiled = {}


def _sched():
    """Pair-half processing order matching W-version DMA arrival.

    Returns a list of (a, v, h): anchor version a (stationary, own 128
    samples), moving version v, column half h. (a,v,h) needs W[v] half h
    and W[a] cols 0:128 (inside h0)."""
    s = []
    for v in range(NV):
        for a in range(v + 1):
            s.append((a, v, 0))
        for b in range(v):
            s.append((v, b, 0))
            s.append((v, b, 1))
        for a in range(v + 1):
            s.append((a, v, 1))
    return s


def _build_kernel(warm=24, warm_free=128, groups_of=4, drow=True,
                  ps_bufs=3):
    from contextlib import ExitStack

    import concourse.bass as bass
    import concourse.tile as tile
    from concourse import bacc, mybir

    fp32 = mybir.dt.float32
    bf16 = mybir.dt.bfloat16
    fp8 = mybir.dt.float8e4
    AX = mybir.AxisListType
    ALU = mybir.AluOpType
    ACTF = mybir.ActivationFunctionType

    nc = bacc.Bacc(
        "TRN2",
        target_bir_lowering=False,
        debug=False,
        enable_asserts=False,
        num_devices=NC_CORES,
    )
    # pre-transposed normalized*32 fp8 embeddings: embT[v*D + d, s] = W_v[s, d]
    embT = nc.dram_tensor("embT", [NV * D, B], fp8, kind="ExternalInput").ap()
    # natural-layout normalized*32 rows of this core's own chunk (for cs reg)
    csrows = nc.dram_tensor("csrows", [NV * P, D], fp8, kind="ExternalInput").ap()
    ratios = nc.dram_tensor("ratios", [P, 1], fp32, kind="ExternalInput").ap()
    eye_d = nc.dram_tensor("eye", [P, P], bf16, kind="ExternalInput").ap()
    out_d = nc.dram_tensor("out", [P, 1], fp32, kind="ExternalOutput").ap()

    sched = _sched()
    ngroups = len(sched) // groups_of
    # h0 slot index per pair (for diag extraction)
    h0_slot = {}
    for slot, (a, v, h) in enumerate(sched):
        if h == 0:
            h0_slot[(a, v)] = slot

    with tile.TileContext(nc) as tc, ExitStack() as ctx:
        consts = ctx.enter_context(tc.tile_pool(name="consts", bufs=1))
        wpool = ctx.enter_context(tc.tile_pool(name="w", bufs=1))
        psum_p = ctx.enter_context(
            tc.tile_pool(name="psum", bufs=ps_bufs, space="PSUM")
        )
        lead_pool = ctx.enter_context(
            tc.tile_pool(name="leadps", bufs=1, space="PSUM")
        )
        esb_p = ctx.enter_context(tc.tile_pool(name="esb", bufs=3))
        csx_p = ctx.enter_context(tc.tile_pool(name="csx", bufs=1))
        scr_p = ctx.enter_context(tc.tile_pool(name="scr", bufs=2))
        dscr_p = ctx.enter_context(tc.tile_pool(name="dscr", bufs=3))
        rsg_p = ctx.enter_context(tc.tile_pool(name="rsg", bufs=1))
        fin_p = ctx.enter_context(tc.tile_pool(name="fin", bufs=1))

        # Pre-load the activation table set holding BOTH Exp and Ln, so the
        # compiler's table-load pass never has to insert a (1.3us) reload
        # between the exp stream and the log/sqrt tail.
        from concourse.hw_specs import get_activation_tables

        tabs = list(get_activation_tables(nc.m.arch).values())
        set_id = next(
            i for i, s in enumerate(tabs)
            if ACTF.Exp in s and ACTF.Ln in s
        )
        nc.scalar.add_instruction(
            mybir.InstLoadActFuncSet(
                name=nc.scalar.bass.get_next_instruction_name(),
                ins=[], outs=[], act_func_set_id=set_id,
            )
        )

        # --- constants / warmup (PE busy from t=0 so the clock is ramped
        # to max by the time the first real matmul's W tile has landed) ---
        ones_sb = consts.tile([P, warm_free], bf16, tag="ones")
        nc.vector.memset(ones_sb, 1.0)
        ones1 = consts.tile([P, 1], fp32, tag="ones1")
        nc.vector.memset(ones1, 1.0)
        eye_sb = consts.tile([P, P], bf16, tag="eye")
        nc.gpsimd.dma_start(out=eye_sb, in_=eye_d)
        r_sb = consts.tile([P, 1], fp32, tag="ratios")
        nc.gpsimd.dma_start(out=r_sb, in_=ratios)

        W = [
            wpool.tile([P, KCH, B], fp8, tag=f"w{v}", name=f"w{v}")
            for v in range(NV)
        ]

        # --- input DMAs (SP/HWDGE, in arrival order the schedule expects).
        # csrows goes first: it is small (fp8) and unblocks the cs-reg chain,
        # which then runs on DVE/Pool during their otherwise-idle prologue.
        # W3's stationary slice (cols 0:128) is pulled forward so (3,b,*)
        # pairs unlock as soon as their moving version is resident.
        def w_dma(v, c0, c1):
            nc.sync.dma_start(
                out=W[v][:, :, c0:c1],
                in_=embT[v * D : (v + 1) * D, c0:c1].rearrange(
                    "(mm p) s -> p mm s", p=P
                ),
            )

        csx = csx_p.tile([P, NV, D], fp8, tag="csx")
        w_dma(0, 0, NT)
        nc.sync.dma_start(
            out=csx[:, 0:2, :],
            in_=csrows[0 : 2 * P, :].rearrange("(v p) d -> p v d", p=P),
        )
        w_dma(0, NT, B)
        w_dma(1, 0, NT)
        w_dma(1, NT, B)
        nc.sync.dma_start(
            out=csx[:, 2:3, :],
            in_=csrows[2 * P : 3 * P, :].rearrange("(v p) d -> p v d", p=P),
        )
        w_dma(2, 0, NT)
        w_dma(2, NT, B)
        nc.sync.dma_start(
            out=csx[:, 3:4, :],
            in_=csrows[3 * P : 4 * P, :].rearrange("(v p) d -> p v d", p=P),
        )
        w_dma(3, 0, NT)
        w_dma(3, NT, B)

        # ---- cs regularization on own chunk: the whole diff chain runs
        # on the otherwise-idle Pool engine during the prologue (u uses a
        # stride-0 broadcast of the ratio column, since tensor-scalar ops
        # are not legal on Pool). The squares+row-sums run on Act as
        # Square-activations with accumulate, bias-gated on the last
        # group's exp output so the scheduler cannot order them ahead of
        # the exp stream. Rows are normalized*SCALE fp8; the 1/SCALE rides
        # the final combine. ----
        e0, k0, etk0, kte0 = (csx[:, vv, :] for vv in range(NV))
        sspack = fin_p.tile([P, 2], fp32, tag="sspack")
        t1 = scr_p.tile([P, D], bf16, tag="cs_t")
        nc.gpsimd.tensor_sub(t1, e0, k0)
        r_bcast = bass.AP(
            tensor=r_sb.tensor, offset=r_sb.offset,
            ap=[r_sb.ap[0], [0, D]],
        )
        u = scr_p.tile([P, D], bf16, tag="cs_u")
        nc.gpsimd.tensor_mul(u, t1, r_bcast)
        d1 = scr_p.tile([P, D], bf16, tag="cs_t")
        nc.gpsimd.tensor_sub(d1, etk0, k0)
        d1m = scr_p.tile([P, D], bf16, tag="cs_d")
        nc.gpsimd.tensor_sub(d1m, d1, u)
        d2 = scr_p.tile([P, D], bf16, tag="cs_t2")
        nc.gpsimd.tensor_sub(d2, kte0, e0)
        d2m = scr_p.tile([P, D], bf16, tag="cs_d2")
        nc.gpsimd.tensor_add(d2m, d2, u)
        # branch-2 square + fold chain entirely on Pool (TT ops are legal
        # there); folded to [P,8], summed into sspack by a tiny gated TTR
        qm2 = scr_p.tile([P, D], bf16, tag="cs_qم2" if False else "cs_qm2")
        nc.gpsimd.tensor_mul(qm2, d2m, d2m)
        qf_prev, width = qm2, D
        while width > 128:
            width //= 2
            qf = scr_p.tile([P, width], bf16, tag=f"cs_qf{width}")
            nc.gpsimd.tensor_add(
                qf, qf_prev[:, 0:width], qf_prev[:, width : 2 * width]
            )
            qf_prev = qf
        qf8 = qf_prev

        dvals = fin_p.tile([P, NV * NV], fp32, tag="dvals")  # [:, a*NV + v]

        # --- main loop over pair-half groups. Fine-grained lead-in groups
        # start the exp stream as soon as W0h0 lands; main groups are
        # 2-half (2 PSUM banks, double-buffered) so PSUM recycles quickly;
        # the four W3h1-gated tail halves write a dedicated tile that
        # reuses the lead tile's banks (free since ~t=11us), so their
        # matmuls run the moment W3h1 arrives, and their exps accumulate
        # their rowsum contributions directly (no DVE tail reduces). ---
        groups = (
            [sched[0:1], sched[1:2], sched[2:4]]
            + [sched[4 + 2 * i : 6 + 2 * i] for i in range(12)]
            + [sched[28 + i : 29 + i] for i in range(4)]
        )
        n_tail = 4
        acc = {}
        acc_last_group = {}
        for gi, chunk in enumerate(groups[: len(groups) - n_tail]):
            for a, v, h in chunk:
                acc_last_group[a] = gi
        rsums_pre = fin_p.tile([P, NV], fp32, tag="rsums_pre")
        rsums_d = fin_p.tile([P, NV], fp32, tag="rsums_d")
        s4s = fin_p.tile([P, NV], fp32, tag="s4s")
        logpack = fin_p.tile([P, 2 * NV], fp32, tag="logpack")
        # self terms dvals[:, a*NV+a]: stride-(NV+1) diagonal view
        dd = bass.AP(
            tensor=dvals.tensor, offset=dvals.offset,
            ap=[dvals.ap[0], [NV + 1, NV]],
        )
        last_h0_group = max(
            gi for gi, chunk in enumerate(groups)
            if any(hh == 0 for _, _, hh in chunk)
        )

        def mk_gate(tag, dval_idx):
            # ==1.0 tile whose data dep pins downstream ops to a known spot
            # in the DVE static order (the Tile scheduler statically orders
            # each engine queue by its own readiness simulation, which
            # models DMAs as instant — ungated, the cs ops would land ahead
            # of main-loop DVE work in the static order and stall it)
            gate = fin_p.tile([P, 1], fp32, tag=tag)
            nc.vector.scalar_tensor_tensor(
                out=gate, in0=dvals[:, dval_idx : dval_idx + 1],
                scalar=0.0, in1=ones1, op0=ALU.mult, op1=ALU.add,
            )
            return gate

        # PSUM: 2-bank lead tile (singles g0/g1) + 2-bank x3 main pool for
        # a 3-deep pipeline that hides the exp->matmul turnaround; the first
        # two tail halves reuse the lead banks the moment W3h1 arrives, the
        # last two rotate through the main pool.
        lead_ps = lead_pool.tile([P, 2, NT], fp32, tag="lead", name="ps_lead")
        first_tail = len(groups) - n_tail
        tail_ps = None
        for i in range(warm):
            nc.tensor.matmul(
                lead_ps[:, i % 2, 0:warm_free],
                ones_sb, ones_sb, start=True, stop=True,
            )
        for g, chunk in enumerate(groups):
            gw = len(chunk)
            tail = g >= first_tail
            if g == 0:
                ps = lead_ps[:, 0:1, :]
            elif g == 1:
                ps = lead_ps[:, 1:2, :]
            elif tail and g - first_tail < 2:
                if tail_ps is None:
                    tail_ps = lead_pool.tile([P, 2, NT], fp32, tag="lead",
                                             name="ps_tail")
                ti = g - first_tail
                ps = tail_ps[:, ti : ti + 1, :]
            elif tail:
                ps2 = psum_p.tile([P, 2, NT], fp32, tag="ps", name="ps_t2")
                ps = ps2[:, 0:1, :]
            else:
                ps = psum_p.tile([P, 2, NT], fp32, tag="ps", name="ps")
            for j, (a, v, h) in enumerate(chunk):
                for m in range(0, KCH, 2):
                    nc.tensor.matmul(
                        ps[:, j, :],
                        W[a][:, m : m + 2, 0:P],
                        W[v][:, m : m + 2, h * NT : (h + 1) * NT],
                        start=(m == 0),
                        stop=(m == KCH - 2),
                        perf_mode=mybir.MatmulPerfMode.DoubleRow,
                    )
            e2 = esb_p.tile([P, gw, NT], bf16, tag=f"e2w{gw}", name="e2")
            if tail:
                a = chunk[0][0]
                nc.scalar.activation(
                    out=e2[:, 0, :], in_=ps[:, 0, :],
                    func=ACTF.Exp, scale=EXPS,
                    accum_out=rsums_d[:, a : a + 1],
                )
                continue
            nc.scalar.activation(
                out=e2.rearrange("p a b -> p (a b)"),
                in_=ps[:, 0:gw, :].rearrange("p a b -> p (a b)"),
                func=ACTF.Exp, scale=EXPS,
            )
            for j, (a, v, h) in enumerate(chunk):
                if a not in acc:
                    acc[a] = rsg_p.tile([P, NT], bf16, tag=f"acc{a}",
                                        name=f"acc{a}")
                    nc.vector.tensor_copy(acc[a], e2[:, j, :])
                else:
                    nc.vector.tensor_add(acc[a], acc[a], e2[:, j, :])
                if h == 0:
                    dscr = dscr_p.tile([P, P], bf16, tag="dscr", name="dscr")
                    nc.vector.scalar_tensor_tensor(
                        out=dscr, in0=e2[:, j, 0:P], scalar=1.0, in1=eye_sb,
                        op0=ALU.mult, op1=ALU.mult,
                        accum_out=dvals[:, a * NV + v : a * NV + v + 1],
                    )
            # per-anchor rowsum partials finish as soon as the anchor's
            # last accumulated half lands — overlaps the remaining groups
            for a in range(NV):
                if acc_last_group.get(a) == g:
                    nc.vector.reduce_sum(
                        out=rsums_pre[:, a : a + 1], in_=acc[a], axis=AX.X
                    )
            h0s = [aa * NV + vv for aa, vv, hh in chunk if hh == 0]
            if g == 7 and h0s:
                # cs branch-1 square: gated to this group's position
                gate5 = mk_gate("gate5", h0s[-1])
                dsq1 = scr_p.tile([P, D], bf16, tag="cs_q1")
                nc.vector.scalar_tensor_tensor(
                    out=dsq1, in0=d1m, scalar=gate5, in1=d1m,
                    op0=ALU.mult, op1=ALU.mult,
                    accum_out=sspack[:, 0:1],
                )
            elif g == 13 and h0s:
                # cs branch-2: sum the Pool-folded squares (tiny gated TTR)
                gate8 = mk_gate("gate8", h0s[-1])
                qdump = fin_p.tile([P, 128], bf16, tag="qdump")
                nc.vector.scalar_tensor_tensor(
                    out=qdump, in0=qf8, scalar=gate8, in1=ones_sb[:, 0:128],
                    op0=ALU.mult, op1=ALU.mult,
                    accum_out=sspack[:, 1:2],
                )
            elif g == last_h0_group:
                # dvals is complete after this group: fold the positives
                nc.vector.reduce_sum(
                    out=s4s,
                    in_=dvals.rearrange("p (a b) -> p a b", a=NV),
                    axis=AX.X,
                )
                nc.vector.tensor_sub(logpack[:, NV : 2 * NV], s4s, dd)  # pos

        # sqrt via exp(0.5*ln(x)): stays inside the ln+exp activation table
        lns = fin_p.tile([P, 2], fp32, tag="lns")
        nc.scalar.activation(out=lns, in_=sspack, func=ACTF.Ln)
        csreg = fin_p.tile([P, 2], fp32, tag="csreg")
        nc.scalar.activation(out=csreg, in_=lns, func=ACTF.Exp, scale=0.5)
        cs_term = fin_p.tile([P, 1], fp32, tag="cs_term")
        nc.vector.reduce_sum(out=cs_term, in_=csreg, axis=AX.X)

        # ---- final reduction ----
        rsums = fin_p.tile([P, NV], fp32, tag="rsums")
        nc.vector.tensor_add(rsums, rsums_pre, rsums_d)
        nc.vector.tensor_sub(logpack[:, 0:NV], rsums, dd)  # denom
        logs = fin_p.tile([P, 2 * NV], fp32, tag="logs")
        nc.scalar.activation(out=logs, in_=logpack, func=ACTF.Ln)
        s12 = fin_p.tile([P, 2], fp32, tag="s12")
        nc.vector.reduce_sum(
            out=s12, in_=logs.rearrange("p (t a) -> p t a", t=2), axis=AX.X
        )
        contrib = fin_p.tile([P, 1], fp32, tag="contrib")
        nc.vector.tensor_sub(contrib, s12[:, 0:1], s12[:, 1:2])
        out_sb = fin_p.tile([P, 1], fp32, tag="out_sb")
        nc.vector.scalar_tensor_tensor(
            out=out_sb, in0=cs_term, scalar=0.5 / SCALE, in1=contrib,
            op0=ALU.mult, op1=ALU.add,
        )
        nc.sync.dma_start(out=out_d, in_=out_sb)

    nc.compile()
    return nc


def _get_nc():
    if "nc" not in _compiled:
        _compiled["nc"] = _build_kernel()
    return _compiled["nc"]


def _make_in_maps(english, etok, ktoe, korean, cs_ratios):
    e = np.asarray(english, dtype=np.float32)
    etk = np.asarray(etok, dtype=np.float32)
    kte = np.asarray(ktoe, dtype=np.float32)
    k = np.asarray(korean, dtype=np.float32)
    r = np.asarray(cs_ratios, dtype=np.float32)

    # version order must match the reference stack: [e, k, etk, kte]
    V4f = np.stack([e, k, etk, kte])  # [4, B, D] fp32
    V4n = V4f / np.linalg.norm(V4f, axis=2, keepdims=True)
    V4s = (V4n * SCALE).astype(ml_dtypes.float8_e4m3)
    eye = np.eye(P, dtype=ml_dtypes.bfloat16)

    in_maps = []
    for c in range(NC_CORES):
        rot = np.roll(V4s, -c * CHUNK, axis=1)  # [4, B, D], own chunk first
        embT = np.ascontiguousarray(rot.transpose(0, 2, 1)).reshape(NV * D, B)
        csrows = np.ascontiguousarray(rot[:, :P, :]).reshape(NV * P, D)
        rr = np.roll(r, -c * CHUNK)[:P].reshape(P, 1).astype(np.float32)
        in_maps.append(
            {"embT": embT, "csrows": csrows, "ratios": rr, "eye": eye}
        )
    return in_maps


def kernel(english, etok, ktoe, korean, cs_ratios):
    from concourse.bass_utils import run_bass_kernel_spmd

    in_maps = _make_in_maps(english, etok, ktoe, korean, cs_ratios)
    nc = _get_nc()
    res = run_bass_kernel_spmd(nc, in_maps, core_ids=list(range(NC_CORES)))
    total = 0.0
    for rmap in res.results:
        total += rmap["out"].astype(np.float64).sum()
    return np.array(total / B, dtype=np.float32)
